# revision 29
# baseline (speedup 1.0000x reference)
"""Trainium2 Bass kernel for nn_AsynBaseStem (sparse 7x7 conv + BN + ReLU +
scatter + 3x3/2 maxpool), 8-core data-parallel over output row bands.

Architecture (per core, fully dense, no indirect DMA):
  - Host prebuilds a [128, 81*646] bf16 operand table T6 per core:
      rows 0..125  : (j,i,ch) j<6 -> fm_pad[r+i, c+j, ch]  (column-shifted planar stripes)
      row  126     : inactive flag (1.0 where pixel has no site, else 0.0)
      row  127     : ones (bias row)
  - Dense conv at every pixel via 2 accumulating matmuls (K=128 main + K=21
    tail read from T6 rows 0..20 at col offset +6). The flag row adds -1e9 to
    inactive pixels (masking), the ones row adds the BN bias.
  - PSUM eviction fuses the column max-pool (DVE even/odd max + ACT third-col
    copy), then a row ring-buffer completes the 3x3/2 max pool.
  - Final ReLU folded into the row pool; one cast-DMA writes [64, p*320] f32;
    the host transposes to [p, q, ch] during unsharding.

kernel(**inputs) takes FULL unsharded inputs, returns [319, 319, 64] f32.
"""
import numpy as np
import ml_dtypes
from contextlib import ExitStack

H = W = 640
CIN, COUT = 3, 64
K, PAD = 7, 3
NCORES = 8
BROWS = 81            # dense rows per core band
WPAD = W + 2 * PAD    # 646
NB = BROWS * WPAD     # T6 free size per core
NBP = NB + 8          # +pad so the tail matmul window (x+6) stays in bounds
PROWS = 40            # pooled rows per core (core 7: 39 valid)
QCOLS = 319
BN_EPS = 1e-5
NEG = -1.0e9


def _build_bass():
    import concourse.bass as bass
    import concourse.mybir as mybir
    import concourse.tile as tile
    from concourse import bacc

    fp32 = mybir.dt.float32
    bf16 = mybir.dt.bfloat16

    nc = bacc.Bacc()
    t6_ext = nc.declare_dram_parameter("t6", [128, NBP], bf16, isOutput=False)
    # packed params: [w | wtail(pad128) | sel126 | sel127 | gam | bet | mu | var]
    par_ext = nc.declare_dram_parameter("par", [128, 8 * COUT], fp32, isOutput=False)
    out_ext = nc.declare_dram_parameter("out", [COUT, PROWS * 320], fp32, isOutput=True)

    with ExitStack() as ctx:
        tc = ctx.enter_context(tile.TileContext(nc))
        cpool = ctx.enter_context(tc.tile_pool(name="const", bufs=1))
        rowp = ctx.enter_context(tc.tile_pool(name="rows", bufs=12))
        ringp = ctx.enter_context(tc.tile_pool(name="ring", bufs=1))
        psp = ctx.enter_context(tc.tile_pool(name="ps", bufs=8, space="PSUM"))

        # ---- weight prep: lhsT A [128, 64] (W'[0:126] + flag row + bias row),
        #      lhsT B [21, 64] (W'[126:147]); W' = W * inv, inv = gamma*rsqrt(var+eps)
        par = cpool.tile([128, 8 * COUT], fp32)
        nc.sync.dma_start(par[:], par_ext[:])
        C = COUT
        wa_f = par[:, 0:C]
        wb_f = par[0:21, C:C + C]
        s126 = par[:, 2 * C:3 * C]
        s127 = par[:, 3 * C:4 * C]
        gam = par[:, 4 * C:5 * C]
        bet = par[:, 5 * C:6 * C]
        mu = par[:, 6 * C:7 * C]
        var = par[:, 7 * C:8 * C]

        # ---- big operand table: chunked load AFTER the small parameter DMAs
        # (HWDGE is FIFO per queue) so weight prep and the first conv rows
        # don't wait for the full 13.4MB stream
        t6 = cpool.tile([128, NBP], bf16)
        bounds = [0, 2] + [2 + 10 * i for i in range(1, 8)] + [BROWS]
        for ck in range(len(bounds) - 1):
            sl = slice(bounds[ck] * WPAD,
                       bounds[ck + 1] * WPAD if ck + 2 < len(bounds) else NBP)
            nc.sync.dma_start(t6[:, sl], t6_ext[:, sl])

        # lhsA = wa*inv + selD*bias' + selN, where selD = sel127 - sel126 and
        # selN = -1e9 at row 126 (host constants); bias' = bet - mu*inv
        inv = cpool.tile([128, COUT], fp32)
        nc.vector.tensor_scalar_add(inv[:], var, BN_EPS)
        nc.scalar.activation(inv[:], inv[:], mybir.ActivationFunctionType.Sqrt)
        nc.vector.reciprocal(inv[:], inv[:])
        nc.vector.tensor_mul(inv[:], inv[:], gam)
        u = cpool.tile([128, COUT], fp32)
        nc.vector.tensor_mul(u[:], mu, inv[:])
        nc.vector.tensor_sub(u[:], bet, u[:])          # u = bias'
        nc.vector.tensor_mul(u[:], u[:], s126)         # u = selD*bias'
        acc = cpool.tile([128, COUT], fp32)
        nc.vector.tensor_mul(acc[:], wa_f, inv[:])
        nc.vector.tensor_add(acc[:], acc[:], s127)     # + selN
        lhsA = cpool.tile([128, COUT], bf16)
        nc.vector.tensor_add(lhsA[:], acc[:], u[:])
        lhsB = cpool.tile([21, COUT], bf16)
        nc.vector.tensor_mul(lhsB[:], wb_f, inv[0:21, :])

        # ---- pooled accumulator [64, PROWS, 320] bf16 and row ring ----
        pooled = ringp.tile([COUT, PROWS * 320], bf16)
        mring = ringp.tile([COUT, 8 * 320], bf16)  # m rows modulo 8

        # Continuous-pixel-space conv: N=512 matmul tiles over x in [0, NB).
        # Row-boundary/pad pixels carry flag=1 -> -1e9, so the pool ignores
        # them. Per-row ev (even cols) and t (pair-max) staging buffers absorb
        # tile fragments; a full-row m then feeds the row pool.
        NT = (NB + 511) // 512
        evrow = {}
        trow = {}

        def finish_row(r):
            mrow = mring[:, (r % 8) * 320:(r % 8) * 320 + 320]
            nc.vector.tensor_tensor(
                out=mrow[:], in0=trow[r][:, 0:320], in1=evrow[r][:, 1:321],
                op=mybir.AluOpType.max)
            del evrow[r], trow[r]
            if r >= 2 and r % 2 == 0:
                p = (r - 2) // 2
                m0 = mring[:, ((r - 2) % 8) * 320:((r - 2) % 8) * 320 + 320]
                m1 = mring[:, ((r - 1) % 8) * 320:((r - 1) % 8) * 320 + 320]
                s01 = rowp.tile([COUT, 320], bf16, tag="s01")
                nc.vector.tensor_tensor(out=s01[:], in0=m0[:], in1=m1[:],
                                        op=mybir.AluOpType.max)
                po = pooled[:, p * 320:(p + 1) * 320]
                nc.vector.scalar_tensor_tensor(
                    out=po[:], in0=s01[:], scalar=0.0, in1=mrow[:],
                    op0=mybir.AluOpType.max, op1=mybir.AluOpType.max)
                # stream pooled rows out in chunks of 10 (cast bf16 -> f32);
                # host does the final [p,q,ch] transpose during unsharding
                if p % 5 == 4:
                    pc = p // 5
                    nc.gpsimd.dma_start(
                        out_ext[:, pc * 1600:(pc + 1) * 1600],
                        pooled[:, pc * 1600:(pc + 1) * 1600])

        for k in range(NT):
            xa = 512 * k
            xb = min(xa + 512, NB)
            wdt = xb - xa
            ps = psp.tile([COUT, 512], fp32, tag="convps")
            nc.tensor.matmul(ps[:, 0:wdt], lhsA[:], t6[0:128, xa:xb],
                             start=True, stop=False)
            nc.tensor.matmul(ps[:, 0:wdt], lhsB[:], t6[0:21, xa + 6:xb + 6],
                             start=False, stop=True)
            for r in range(xa // WPAD, (xb - 1) // WPAD + 1):
                ca = max(xa, r * WPAD) - r * WPAD     # even
                cb = min(xb, r * WPAD + WPAD) - r * WPAD  # even
                if r not in evrow:
                    evrow[r] = rowp.tile([COUT, 324], bf16, tag="evrow", name=f"evrow{r}")
                    trow[r] = rowp.tile([COUT, 324], bf16, tag="trowb", name=f"trowb{r}")
                ne = (cb - ca) // 2
                p0 = r * WPAD + ca - xa               # psum-local offset
                nc.scalar.copy(evrow[r][:, ca // 2:ca // 2 + ne],
                               ps[:, p0:p0 + 2 * ne:2])
                nc.vector.tensor_tensor(
                    out=trow[r][:, ca // 2:ca // 2 + ne],
                    in0=evrow[r][:, ca // 2:ca // 2 + ne],
                    in1=ps[:, p0 + 1:p0 + 2 * ne:2],
                    op=mybir.AluOpType.max)
                if cb == WPAD:
                    finish_row(r)



    nc.finalize()
    return nc


_NC_CACHE = None


def _get_nc():
    global _NC_CACHE
    if _NC_CACHE is None:
        _NC_CACHE = _build_bass()
    return _NC_CACHE


def build_in_maps(update_location, feature_map, weight, gamma, beta,
                  running_mean, running_var):
    fm = np.asarray(feature_map, np.float32)
    loc = np.asarray(update_location).astype(np.int64)
    wt = np.asarray(weight, np.float32)

    fm_pad = np.pad(fm, ((PAD, PAD), (PAD, PAD), (0, 0)))          # [646,646,3]
    # stripes B_T[t=(i,ch), r, c] = fm_pad[r+i, c, ch], r in 0..640 (row 640 pad)
    bt = np.zeros((21, H + 1, WPAD), np.float32)
    for i in range(K):
        for ch in range(CIN):
            bt[i * CIN + ch, 0:H, :] = fm_pad[i:i + H, :, ch]
    bt = bt.astype(ml_dtypes.bfloat16)

    # inactive flag = 1 where no site; indexed by output pixel (r, c) at
    # position c in the 646-pitch row; columns 640..645 stay inactive.
    flag = np.ones((H + 1, WPAD), np.float32)
    flag[loc[:, 0], loc[:, 1]] = 0.0
    flag[:, H:] = 1.0
    flag = flag.astype(ml_dtypes.bfloat16)

    # reordered weights W_re[(j,i,ch), o] = weight[i, j, ch, o]
    w_re = np.ascontiguousarray(
        wt.transpose(1, 0, 2, 3).reshape(147, COUT)).astype(np.float32)

    bcast = lambda v: np.ascontiguousarray(
        np.broadcast_to(np.asarray(v, np.float32)[None, :], (128, COUT)))

    in_maps = []
    for k in range(NCORES):
        r0 = 80 * k
        t6 = np.zeros((128, BROWS, WPAD), ml_dtypes.bfloat16)
        for j in range(6):
            sl = bt[:, r0:r0 + BROWS, :]
            t6[j * 21:(j + 1) * 21, :, :-j or None] = sl[:, :, j:]
        t6[126] = flag[r0:r0 + BROWS]
        t6[127] = np.ones((BROWS, WPAD), ml_dtypes.bfloat16)
        wfull = np.zeros((128, COUT), np.float32)
        wfull[0:126] = w_re[0:126]
        sel126 = np.zeros((128, COUT), np.float32)   # selD: +1 at row 127, -1 at 126
        sel126[127] = 1.0
        sel126[126] = -1.0
        sel127 = np.zeros((128, COUT), np.float32)   # selN: NEG at row 126
        sel127[126] = NEG
        t6p = np.zeros((128, NBP), ml_dtypes.bfloat16)
        t6p[:, :NB] = t6.reshape(128, NB)
        wtail = np.zeros((128, COUT), np.float32)
        wtail[0:21] = w_re[126:147]
        par = np.concatenate([wfull, wtail, sel126, sel127, bcast(gamma),
                              bcast(beta), bcast(running_mean),
                              bcast(running_var)], axis=1)
        in_maps.append({"t6": t6p, "par": np.ascontiguousarray(par)})
    return in_maps


def kernel(update_location, feature_map, weight, gamma, beta, running_mean,
           running_var):
    from concourse.bass_utils import run_bass_kernel_spmd

    in_maps = build_in_maps(update_location, feature_map, weight, gamma, beta,
                            running_mean, running_var)
    nc = _get_nc()
    res = run_bass_kernel_spmd(nc, in_maps, core_ids=list(range(NCORES)))
    # per-core out is [64, PROWS*320] f32 (ch-major); assemble [319, 319, 64]
    parts = []
    for k in range(NCORES):
        o = res.results[k]["out"].reshape(COUT, PROWS, 320)
        parts.append(o.transpose(1, 2, 0)[:, :QCOLS, :])
    out = np.concatenate(parts, axis=0)[:QCOLS]
    return np.ascontiguousarray(out).astype(np.float32)
